# revision 29
# baseline (speedup 1.0000x reference)
"""Trainium2 Bass kernel for the CapacityNN PINN forward pass.

Computes, for N = B*S collocation points x = (s, t):
  U   = MLP([s_norm, t_norm]) * tgt_std + tgt_mean
  F   = U_t  - G(U)             (G = Verhulst logistic growth term)
  F_t = U_tt - G'(U) * U_t
where U_t/U_tt are 1st/2nd derivatives w.r.t. t_norm, computed exactly by
forward-mode Taylor (jet) propagation through the tanh MLP.

Sharding: pure data parallel over 8 NeuronCores (8192 points/core),
MLP weights + PDE scalars replicated. All math runs on-device; the host
only reorders data (transposes for layout, shard/gather).

Device layout: hidden dim (256) on partitions (2 tiles of 128), points on
the free dim, chunks of 1024 points. Three fp16 streams per layer:
  Hv (values), H1 (d/dt_norm tangent), H2 (half the 2nd tangent; the 1/2
  is folded into the layer-1 H2 weights and restored in the final lt4).
Jet recurrence per layer (Z* = psum matmul results):
  av  = tanh(Z0 + b)            [ACT]
  z1c = copy(Z1) -> sbuf fp16   [ACT]
  ee  = av*av                   [Pool]
  dm  = 1 - ee                  [DVE 4x tensor_scalar]
  st  = z1c*z1c                 [DVE 2x]
  tt  = av*st                   [Pool/DVE split]
  qt  = Z2 - tt                 [DVE, fused psum evacuation]
  h1t = dm*z1c                  [DVE 2x]
  h2t = dm*qt                   [DVE/Pool split]
Engine assignment is tuned so ACT/DVE/Pool all sit near the PE roofline.
"""

import os
import sys
import tempfile

import numpy as np

for _p in ("/opt/trn_rl_repo", "/root/.axon_site/_ro/trn_rl_repo"):
    if os.path.isdir(_p) and _p not in sys.path:
        sys.path.insert(0, _p)

import concourse.bass as bass
import concourse.bacc as bacc
import concourse.tile as tile
from concourse import mybir
from concourse.bass_utils import run_bass_kernel_spmd

AF = mybir.ActivationFunctionType
OP = mybir.AluOpType
F32 = mybir.dt.float32
F32R = mybir.dt.float32r
F16 = mybir.dt.float16

NCORES = 8
B, S, H = 512, 128, 256
N = B * S                  # 65536 points
NLOC = N // NCORES         # 8192 points per core
CH = 1024                  # points per on-chip chunk
NCHUNK = NLOC // CH
PT = CH // 512             # 512-wide matmul point tiles per chunk
PPP = NLOC // 128          # points per partition in the tail layout (64)
PG = 512                   # points per PSUM group (1 bank)
NG = CH // PG
SQRT2 = float(np.sqrt(2.0))


def _build():
    nc = bacc.Bacc(
        "TRN2",
        target_bir_lowering=False,
        debug=False,
        enable_asserts=False,
        num_devices=NCORES,
    )

    def din(name, shape, dt=F32):
        return nc.dram_tensor(name, list(shape), dt, kind="ExternalInput").ap()

    x2 = din("x2", (2, NLOC), F32R)            # rows: raw s, raw t (per-core slice)
    w0ts_d = din("w0ts", (2, H), F32R)         # (W0/std).T, std folded on host
    bt0 = din("bt0", (H,))                     # b0 - W0 @ (mean/std)
    wth = {l: din(f"w{l}th", (H, H), F16) for l in (1, 2, 3)}  # fp16 Wl.T
    w1wt = din("w1wt", (H, H), F16)     # (W1*diag(w0c1)).T fp16
    w1w2t = din("w1w2t", (H, H), F16)   # (W1*diag(-w0c1^2)).T fp16 (half conv)
    lt4h = din("lt4h", (6, 128, 3), F16)   # block-diag final lhsT, H2 col x2
    negid = din("negid", (128, 128), F16)  # -I for psum-accumulated subtraction
    bs = {l: din(f"b{l}", (H,)) for l in (1, 2, 3)}
    # host-folded PDE/tail scalars
    ct = din("ct", (1,))        # C
    nr_d = din("nr", (1,))      # -r
    c1_d = din("c1", (1,))      # -1/(K-C)
    mc3_d = din("mc3", (1,))    # 2r/(K-C)
    tmb_d = din("tmb", (1,))    # b4*tgt_std + tgt_mean
    sts_d = din("sts", (1,))    # tgt_std
    out = nc.dram_tensor("out", [3, NLOC], F32, kind="ExternalOutput").ap()

    def pool_mul(dst, a, b):
        nc.gpsimd.tensor_tensor(dst, a, b, OP.mult)

    def dve_mul(dst, a, b):
        nc.vector.tensor_tensor(dst, a, b, OP.mult)

    with tile.TileContext(nc) as tc:
        from contextlib import ExitStack

        with ExitStack() as ctx:
            const = ctx.enter_context(tc.tile_pool(name="const", bufs=1))
            sb = ctx.enter_context(tc.tile_pool(name="sb", bufs=1))
            ps = ctx.enter_context(tc.tile_pool(name="ps", bufs=1, space="PSUM"))

            # ---------- one-time prep ----------
            # DMA queue discipline: nc.sync is reserved for the x2/tp/out
            # streams so chunk-0's input issues immediately; const loads ride
            # the four engine queues in consumption order (L0-critical first,
            # tail-only scalars last).
            _qs = [nc.scalar]
            _qi = [0]

            def qdma(outt, inap):
                _qs[_qi[0] % len(_qs)].dma_start(out=outt, in_=inap)
                _qi[0] += 1

            def new1(name):
                return const.tile([128, 1], F32, name=name)

            def bc_tile2(src_ap, name):
                t = const.tile([128, 2], F32, name=name)
                qdma(t, bass.AP(src_ap.tensor, 0, [[0, 128], [1, 2]]))
                return t

            def bc_tile(src_ap, off, name):
                t = new1(name)
                qdma(t, bass.AP(src_ap.tensor, off, [[0, 128], [1, 1]]))
                return t

            # --- L0-critical loads (all folding done on host) ---
            w0ts = const.tile([2, H], F32R, name="w0ts")
            qdma(w0ts, w0ts_d)
            beta0 = []
            for m in range(2):
                t = new1(f"beta_{m}")
                qdma(t, bass.AP(bt0.tensor, 128 * m, [[1, 128], [1, 1]]))
                beta0.append(t)

            # --- prewarm the ACT function tables (Tanh/Copy/Square) on a
            # dummy tile so the 1.3us table loads overlap the input DMAs ---
            warm = const.tile([1, 1], F32, name="warm")
            nc.vector.memset(warm, 0.0)
            warm2 = const.tile([1, 1], F16, name="warm2")
            nc.scalar.activation(warm2, warm, AF.Tanh)
            nc.scalar.activation(warm2, warm, AF.Copy)
            nc.scalar.activation(warm2, warm, AF.Square)

            # --- preload the first pair's inputs so they beat the bulk
            # const loads into the sync DMA queue ---
            x2_pre = {}
            for c in (0, 1):
                t = sb.tile([2, CH], F32R, tag="x2c", bufs=4)
                nc.sync.dma_start(out=t, in_=x2[:, c * CH : (c + 1) * CH])
                x2_pre[c] = t

            # --- hidden-layer weights: batched [128, 256] loads, layer order,
            # on the sync queue (ACT queue stays clear for tanh) ---
            def wload(name, srcd):
                halves = []
                for kk in range(2):
                    t = const.tile([128, H], F16, name=f"{name}_{kk}")
                    nc.sync.dma_start(
                        out=t,
                        in_=bass.AP(srcd.tensor, kk * 128 * H, [[H, 128], [1, H]]),
                    )
                    halves.append([t[:, 0:128], t[:, 128:256]])
                return halves

            bl = {}

            def bload(l):
                bl[l] = []
                for m in range(2):
                    t = new1(f"bl{l}_{m}")
                    nc.sync.dma_start(
                        out=t, in_=bass.AP(bs[l].tensor, 128 * m, [[1, 128], [1, 1]])
                    )
                    bl[l].append(t)

            wt16 = {}
            wtw = wload("wtw", w1wt)
            wtw2 = wload("wtw2", w1w2t)
            wt16[1] = wload("wth1", wth[1])
            bload(1)
            wt16[2] = wload("wth2", wth[2])
            bload(2)
            wt16[3] = wload("wth3", wth[3])
            bload(3)

            # final-projection block-diag lhsT: one [128, 18] load
            lt18 = const.tile([128, 18], F16, name="lt18")
            nc.sync.dma_start(
                out=lt18, in_=bass.AP(lt4h.tensor, 0, [[3, 128], [128 * 3, 6], [1, 3]])
            )
            lt4 = [
                [lt18[:, (2 * s + kk) * 3 : (2 * s + kk) * 3 + 3] for kk in range(2)]
                for s in range(3)
            ]
            nid = const.tile([128, 128], F16, name="nid")
            nc.sync.dma_start(out=nid, in_=negid)



            # ---------- main loop: chunks processed in interleaved pairs ----------
            # PE order alternates chunk A and B at each stage (L0..L3, FIN) so
            # every elementwise chain of one chunk is covered by the other
            # chunk's matmul stream; the PE never waits and stays at full
            # pstate. The -I/-h2t accumulations are deferred to the next
            # stage's flush point (always the other chunk's).
            y3f = sb.tile([3, NLOC], F32, name="y3f")
            tp = sb.tile([128, 3 * PPP], F32, name="tp")
            pend = []

            def flush_pend():
                while pend:
                    pend.pop(0)()

            def load_x2(c):
                x2c = sb.tile([2, CH], F32R, tag="x2c", bufs=4)
                nc.sync.dma_start(out=x2c, in_=x2[:, c * CH : (c + 1) * CH])
                return x2c

            def layer0(st_c, x2c):
                Hv, H1, H2 = st_c
                for m in range(2):
                    av = sb.tile([128, CH], F16, tag=f"hv{m}", bufs=4, name="av")
                    for g in range(NG):
                        sl = slice(g * PG, (g + 1) * PG)
                        pz0 = ps.tile([128, PG], F32, tag="pz0", bufs=2, name="pz0")
                        nc.tensor.matmul(
                            pz0,
                            w0ts[:, m * 128 : (m + 1) * 128],
                            x2c[:, sl],
                            start=True,
                            stop=True,
                        )
                        nc.scalar.activation(av[:, sl], pz0, AF.Tanh, beta0[m])
                    if m == 0:
                        flush_pend()
                    ee = sb.tile([128, CH], F16, tag=f"ee{m}", bufs=3, name="ee")
                    dve_mul(ee, av, av)
                    d0 = sb.tile([128, CH], F16, tag=f"h1{m}", bufs=4, name="d0")
                    nc.vector.tensor_scalar(d0, ee, -1.0, 1.0, OP.mult, OP.add)
                    ad = sb.tile([128, CH], F16, tag=f"h2{m}", bufs=4, name="ad")
                    dve_mul(ad, d0, av)
                    Hv[m], H1[m], H2[m] = av, d0, ad

            def layer(st_c, l):
                Hv, H1, H2 = st_c
                W1h = wtw if l == 1 else wt16[l]
                W2h = wtw2 if l == 1 else wt16[l]
                nHv = [None] * 2
                nH1 = [None] * 2
                nH2 = [None] * 2
                for m in range(2):
                    av = sb.tile([128, CH], F16, tag=f"hv{m}", bufs=4, name="av")
                    z1c = sb.tile([128, CH], F16, tag=f"z1c{m}", bufs=3, name="z1c")
                    for g in range(NG):
                        sl = slice(g * PG, (g + 1) * PG)
                        pz0 = ps.tile([128, PG], F32, tag="pz0", bufs=2, name="pz0")
                        for kk in range(2):
                            nc.tensor.matmul(
                                pz0,
                                wt16[l][kk][m],
                                Hv[kk][:, sl],
                                start=(kk == 0),
                                stop=(kk == 1),
                            )
                        nc.scalar.activation(av[:, sl], pz0, AF.Tanh, bl[l][m])
                    if m == 0:
                        flush_pend()
                    for g in range(NG):
                        sl = slice(g * PG, (g + 1) * PG)
                        pz1 = ps.tile([128, PG], F32, tag="pz1", bufs=2, name="pz1")
                        for kk in range(2):
                            nc.tensor.matmul(
                                pz1,
                                W1h[kk][m],
                                H1[kk][:, sl],
                                start=(kk == 0),
                                stop=(kk == 1),
                            )
                        nc.scalar.activation(z1c[:, sl], pz1, AF.Copy)
                    ee = sb.tile([128, CH], F16, tag=f"ee{m}", bufs=3, name="ee")
                    pool_mul(ee, av, av)
                    dm = sb.tile([128, CH], F16, tag=f"dm{m}", bufs=3, name="dm")
                    nc.vector.tensor_scalar(dm, ee, -1.0, 1.0, OP.mult, OP.add)
                    st = sb.tile([128, CH], F16, tag=f"st{m}", bufs=3, name="st")
                    if l == 1:
                        nc.scalar.activation(st, z1c, AF.Square)
                    else:
                        dve_mul(st, z1c, z1c)
                    tt = sb.tile([128, CH], F16, tag=f"tt{m}", bufs=3, name="tt")
                    dve_mul(tt, av, st)
                    h1t = sb.tile([128, CH], F16, tag=f"h1{m}", bufs=4, name="h1t")
                    dve_mul(h1t, dm, z1c)
                    # second-tangent partial sums; -tt matmul + evacuation
                    # deferred to the next stage's flush point. pz2 is a
                    # 2-bank tile so the evacuation runs 1024-wide.
                    h2t = sb.tile([128, CH], F16, tag=f"h2{m}", bufs=4, name="h2t")
                    pz2 = ps.tile([128, CH], F32, tag="pz2", bufs=2, name="pz2")
                    for g in range(NG):
                        sl = slice(g * PG, (g + 1) * PG)
                        for kk in range(2):
                            nc.tensor.matmul(
                                pz2[:, sl],
                                W2h[kk][m],
                                H2[kk][:, sl],
                                start=(kk == 0),
                                stop=False,
                            )

                    def fin(pz2=pz2, tt=tt, dm=dm, h2t=h2t):
                        for g in range(NG):
                            sl = slice(g * PG, (g + 1) * PG)
                            nc.tensor.matmul(
                                pz2[:, sl], nid, tt[:, sl], start=False, stop=True
                            )
                        nc.vector.tensor_tensor(h2t, pz2, dm, OP.mult)

                    pend.append(fin)
                    nHv[m], nH1[m], nH2[m] = av, h1t, h2t
                st_c[0], st_c[1], st_c[2] = nHv, nH1, nH2

            def final_proj(st_c, c):
                Hv, H1, H2 = st_c
                for i in range(PT):
                    pyt = ps.tile([128, PG], F32, tag="pz1", bufs=2, name="pyt")
                    py = pyt[0:3, :]
                    first = True
                    for s_idx, stream in enumerate((Hv, H1, H2)):
                        for kk in range(2):
                            nc.tensor.matmul(
                                py,
                                lt4[s_idx][kk],
                                stream[kk][:, i * 512 : (i + 1) * 512],
                                start=first,
                                stop=(s_idx == 2 and kk == 1),
                            )
                            first = False
                    if i == 0:
                        flush_pend()
                    nc.scalar.copy(
                        y3f[:, c * CH + i * 512 : c * CH + (i + 1) * 512], py
                    )
                for s_idx in range(3):
                    nc.sync.dma_start(
                        out=tp[c * 16 : (c + 1) * 16,
                               s_idx * PPP : (s_idx + 1) * PPP],
                        in_=y3f[s_idx : s_idx + 1, c * CH : (c + 1) * CH],
                    )

            # Software-pipelined pair loop. The next pair's L0 stages are
            # emitted between the FINs so the PE stream never drains at a
            # pair boundary.
            npairs = NCHUNK // 2
            nxt = {}
            for cp in range(npairs):
                A, Bc = 2 * cp, 2 * cp + 1
                for cn in (2 * cp + 2, 2 * cp + 3):
                    if cn < NCHUNK:
                        x2_pre[cn] = load_x2(cn)
                if cp == 0:
                    stA = [[None] * 2, [None] * 2, [None] * 2]
                    stB = [[None] * 2, [None] * 2, [None] * 2]
                    layer0(stA, x2_pre[A])
                    layer0(stB, x2_pre[Bc])
                else:
                    stA, stB = nxt["A"], nxt["B"]
                for l in (1, 2, 3):
                    layer(stA, l)
                    layer(stB, l)
                final_proj(stA, A)
                if cp + 1 < npairs:
                    nA = [[None] * 2, [None] * 2, [None] * 2]
                    layer0(nA, x2_pre[2 * cp + 2])
                    nxt["A"] = nA
                final_proj(stB, Bc)
                if cp + 1 < npairs:
                    nB = [[None] * 2, [None] * 2, [None] * 2]
                    layer0(nB, x2_pre[2 * cp + 3])
                    nxt["B"] = nB

            # --- tail scalars (host-folded) ---
            C_t = bc_tile(ct, 0, "bc_ct")
            nr = bc_tile(nr_d, 0, "bc_nr")
            c1 = bc_tile(c1_d, 0, "bc_c1")
            mc3 = bc_tile(mc3_d, 0, "bc_mc3")
            tmb = bc_tile(tmb_d, 0, "bc_tmb")
            sts = bc_tile(sts_d, 0, "bc_sts")
            bc_ts = sts

            # ----- tail (once): PDE algebra on the [128, PPP] layout -----
            yv = tp[:, 0:PPP]
            yt = tp[:, PPP : 2 * PPP]
            ytt = tp[:, 2 * PPP : 3 * PPP]
            oc = sb.tile([128, 3 * PPP], F32, name="oc")
            U = oc[:, 0:PPP]
            Fo = oc[:, PPP : 2 * PPP]
            Ft = oc[:, 2 * PPP : 3 * PPP]

            def tl(name):
                return sb.tile([128, PPP], F32, name=name)

            ut, utt, vv, v2, w1, q1, t1 = (
                tl("ut"), tl("utt"), tl("vv"), tl("v2"), tl("w1"), tl("q1"), tl("t1"),
            )
            nc.vector.tensor_scalar(U, yv, bc_ts, tmb, OP.mult, OP.add)
            nc.vector.tensor_scalar(ut, yt, sts, None, OP.mult)
            nc.vector.tensor_scalar(utt, ytt, sts, None, OP.mult)
            nc.vector.tensor_scalar(vv, U, C_t, None, OP.subtract)
            nc.vector.tensor_tensor(v2, vv, vv, OP.mult)
            nc.vector.scalar_tensor_tensor(w1, v2, c1, vv, OP.mult, OP.add)
            nc.vector.scalar_tensor_tensor(Fo, w1, nr, ut, OP.mult, OP.add)
            nc.vector.tensor_tensor(q1, vv, ut, OP.mult)
            nc.vector.scalar_tensor_tensor(t1, ut, nr, utt, OP.mult, OP.add)
            nc.vector.scalar_tensor_tensor(Ft, q1, mc3, t1, OP.mult, OP.add)
            for s_idx, srcap in enumerate((U, Fo, Ft)):
                nc.sync.dma_start(out=out[s_idx : s_idx + 1, :], in_=srcap)






    nc.compile()
    return nc


_STATE = {}


def _get_nc():
    if "nc" not in _STATE:
        _STATE["nc"] = _build()
    return _STATE["nc"]


def _make_lt4(w4):
    # final-projection block-diag lhsT; H2 stream carries h2/2, restore x2
    scales = (1.0, 1.0, 2.0)
    out = np.zeros((6, 128, 3), np.float32)
    for s_idx in range(3):
        for kk in range(2):
            out[2 * s_idx + kk, :, s_idx] = (
                scales[s_idx] * w4[0, kk * 128 : (kk + 1) * 128]
            )
    return out


def _prep_in_maps(inputs):
    f = np.float32

    def arr(k):
        return np.ascontiguousarray(np.asarray(inputs[k], f))

    x = np.asarray(inputs["inputs"], f).reshape(N, 2)
    istd = arr("in_std") + np.float32(1e-8)
    w0s = arr("W0") / istd[None, :]
    bt0 = arr("b0") - arr("W0") @ (arr("in_mean") / istd)
    r = np.exp(-arr("log_growth_rate"))
    K = np.float32(0.2) + np.float32(0.8) / (1 + np.exp(-arr("log_carrying_capacity")))
    C = np.float32(0.1) / (1 + np.exp(-arr("log_initial_loss")))
    ts1 = arr("tgt_std").reshape(1)
    shared = {
        "w0ts": np.ascontiguousarray(w0s.T),
        "bt0": np.ascontiguousarray(bt0),
        "lt4h": _make_lt4(arr("W4").reshape(1, H)).astype(np.float16),
        "w1th": np.ascontiguousarray(arr("W1").T).astype(np.float16),
        "w1wt": np.ascontiguousarray(
            (arr("W1") * arr("W0")[:, 1][None, :]).T
        ).astype(np.float16),
        "w1w2t": np.ascontiguousarray(
            (arr("W1") * (-(arr("W0")[:, 1] ** 2))[None, :]).T
        ).astype(np.float16),
        "negid": (-np.eye(128)).astype(np.float16),
        "w2th": np.ascontiguousarray(arr("W2").T).astype(np.float16),
        "w3th": np.ascontiguousarray(arr("W3").T).astype(np.float16),
        "b1": arr("b1"),
        "b2": arr("b2"),
        "b3": arr("b3"),
        "ct": np.asarray(C, f).reshape(1),
        "nr": np.asarray(-r, f).reshape(1),
        "c1": np.asarray(-1.0 / (K - C), f).reshape(1),
        "mc3": np.asarray(2.0 * r / (K - C), f).reshape(1),
        "tmb": np.asarray(
            arr("b4").reshape(1) * ts1 + arr("tgt_mean").reshape(1), f
        ).reshape(1),
        "sts": np.asarray(ts1, f).reshape(1),
    }
    in_maps = []
    for c in range(NCORES):
        m = dict(shared)
        m["x2"] = np.ascontiguousarray(x[c * NLOC : (c + 1) * NLOC].T)
        in_maps.append(m)
    return in_maps


def run(inputs, trace=False):
    nc = _get_nc()
    in_maps = _prep_in_maps(inputs)
    kw = {}
    if trace:
        kw["tmpdir"] = tempfile.mkdtemp(prefix="bassk_prof_")
    res = run_bass_kernel_spmd(
        nc, in_maps, core_ids=list(range(NCORES)), trace=trace, **kw
    )
    U = np.empty((N,), np.float32)
    F = np.empty((N,), np.float32)
    Ft = np.empty((N,), np.float32)
    for c in range(NCORES):
        o = res.results[c]["out"]
        U[c * NLOC : (c + 1) * NLOC] = o[0]
        F[c * NLOC : (c + 1) * NLOC] = o[1]
        Ft[c * NLOC : (c + 1) * NLOC] = o[2]
    shp = (B, S, 1)
    return (U.reshape(shp), F.reshape(shp), Ft.reshape(shp)), res


def kernel(**inputs):
    outs, _ = run(inputs, trace=False)
    return outs


# ---------------------------------------------------------------------------
# Dev-loop timing: persistent jitted executable (mirrors
# bass2jax.run_bass_via_pjrt's multi-core branch) so repeated executions
# reuse one compiled NEFF and can be timed back-to-back.
# ---------------------------------------------------------------------------
def _make_runner():
    if "runner" in _STATE:
        return _STATE["runner"]
    import jax
    from jax.experimental.shard_map import shard_map
    from jax.sharding import Mesh, PartitionSpec
    from concourse import bass2jax

    bass2jax.install_neuronx_cc_hook()
    nc = _get_nc()

    in_names, out_names, out_avals, zero_outs = [], [], [], []
    for alloc in nc.m.functions[0].allocations:
        if not isinstance(alloc, mybir.MemoryLocationSet):
            continue
        name = alloc.memorylocations[0].name
        if alloc.kind == "ExternalInput":
            if nc.partition_id_tensor is None or name != nc.partition_id_tensor.name:
                in_names.append(name)
        elif alloc.kind == "ExternalOutput":
            out_names.append(name)
            shape = tuple(alloc.tensor_shape)
            dtype = mybir.dt.np(alloc.dtype)
            out_avals.append(jax.core.ShapedArray(shape, dtype))
            zero_outs.append(np.zeros(shape, dtype))
    n_params = len(in_names)
    n_outs = len(out_avals)
    all_names = in_names + out_names
    if nc.partition_id_tensor is not None:
        all_names = all_names + [nc.partition_id_tensor.name]

    def _body(*args):
        operands = list(args)
        if nc.partition_id_tensor is not None:
            operands.append(bass2jax.partition_id_tensor())
        outs = bass2jax._bass_exec_p.bind(
            *operands,
            out_avals=tuple(out_avals),
            in_names=tuple(all_names),
            out_names=tuple(out_names),
            lowering_input_output_aliases=(),
            sim_require_finite=True,
            sim_require_nnan=True,
            nc=nc,
        )
        return tuple(outs)

    devices = jax.devices()[:NCORES]
    mesh = Mesh(np.asarray(devices), ("core",))
    donate = tuple(range(n_params, n_params + n_outs))
    sharded = jax.jit(
        shard_map(
            _body,
            mesh=mesh,
            in_specs=(PartitionSpec("core"),) * (n_params + n_outs),
            out_specs=(PartitionSpec("core"),) * n_outs,
            check_rep=False,
        ),
        donate_argnums=donate,
        keep_unused=True,
    )
    _STATE["runner"] = (sharded, in_names, out_names, out_avals, zero_outs)
    return _STATE["runner"]


def run_timed(inputs, iters=20):
    """Run via a persistent executable; return (outputs, per_iter_ns)."""
    import time as _time

    import jax

    sharded, in_names, out_names, out_avals, zero_outs = _make_runner()
    in_maps = _prep_in_maps(inputs)
    concat_in = [
        np.concatenate([np.asarray(in_maps[c][n]) for c in range(NCORES)], axis=0)
        for n in in_names
    ]
    dev_in = [jax.device_put(a) for a in concat_in]

    def zeros():
        return [
            np.zeros((NCORES * z.shape[0], *z.shape[1:]), z.dtype) for z in zero_outs
        ]

    # warmup (compiles on first call)
    outs = sharded(*dev_in, *zeros())
    jax.block_until_ready(outs)
    out_np = [np.asarray(o) for o in outs]

    zbufs = [zeros() for _ in range(iters)]
    t0 = _time.perf_counter()
    last = None
    for i in range(iters):
        last = sharded(*dev_in, *zbufs[i])
    jax.block_until_ready(last)
    t1 = _time.perf_counter()
    per_iter_ns = (t1 - t0) / iters * 1e9

    per_core = [
        {
            name: out_np[i].reshape(NCORES, *out_avals[i].shape)[c]
            for i, name in enumerate(out_names)
        }
        for c in range(NCORES)
    ]
    U = np.empty((N,), np.float32)
    F = np.empty((N,), np.float32)
    Ft = np.empty((N,), np.float32)
    for c in range(NCORES):
        o = per_core[c]["out"]
        U[c * NLOC : (c + 1) * NLOC] = o[0]
        F[c * NLOC : (c + 1) * NLOC] = o[1]
        Ft[c * NLOC : (c + 1) * NLOC] = o[2]
    shp = (B, S, 1)
    return (U.reshape(shp), F.reshape(shp), Ft.reshape(shp)), per_iter_ns


# revision 41
# speedup vs baseline: 1.0129x; 1.0129x over previous
"""Trainium2 Bass kernel for the CapacityNN PINN forward pass.

Computes, for N = B*S collocation points x = (s, t):
  U   = MLP([s_norm, t_norm]) * tgt_std + tgt_mean
  F   = U_t  - G(U)             (G = Verhulst logistic growth term)
  F_t = U_tt - G'(U) * U_t
where U_t/U_tt are 1st/2nd derivatives w.r.t. t_norm, computed exactly by
forward-mode Taylor (jet) propagation through the tanh MLP.

Sharding: pure data parallel over 8 NeuronCores (8192 points/core),
MLP weights + PDE scalars replicated. All math runs on-device; the host
only reorders data (transposes for layout, shard/gather).

Device layout: hidden dim (256) on partitions (2 tiles of 128), points on
the free dim, chunks of 1024 points. Three fp16 streams per layer:
  Hv (values), H1 (d/dt_norm tangent), H2 (half the 2nd tangent; the 1/2
  is folded into the layer-1 H2 weights and restored in the final lt4).
Jet recurrence per layer (Z* = psum matmul results):
  av  = tanh(Z0 + b)            [ACT]
  z1c = copy(Z1) -> sbuf fp16   [ACT]
  ee  = av*av                   [Pool]
  dm  = 1 - ee                  [DVE 4x tensor_scalar]
  st  = z1c*z1c                 [DVE 2x]
  tt  = av*st                   [Pool/DVE split]
  qt  = Z2 - tt                 [DVE, fused psum evacuation]
  h1t = dm*z1c                  [DVE 2x]
  h2t = dm*qt                   [DVE/Pool split]
Engine assignment is tuned so ACT/DVE/Pool all sit near the PE roofline.
"""

import os
import sys
import tempfile

import numpy as np

for _p in ("/opt/trn_rl_repo", "/root/.axon_site/_ro/trn_rl_repo"):
    if os.path.isdir(_p) and _p not in sys.path:
        sys.path.insert(0, _p)

import concourse.bass as bass
import concourse.bacc as bacc
import concourse.tile as tile
from concourse import mybir
from concourse.bass_utils import run_bass_kernel_spmd

AF = mybir.ActivationFunctionType
OP = mybir.AluOpType
F32 = mybir.dt.float32
F32R = mybir.dt.float32r
F16 = mybir.dt.float16

NCORES = 8
B, S, H = 512, 128, 256
N = B * S                  # 65536 points
NLOC = N // NCORES         # 8192 points per core
CH = 1024                  # points per on-chip chunk
NCHUNK = NLOC // CH
PT = CH // 512             # 512-wide matmul point tiles per chunk
PPP = NLOC // 128          # points per partition in the tail layout (64)
PG = 512                   # points per PSUM group (1 bank)
NG = CH // PG
SQRT2 = float(np.sqrt(2.0))


def _build():
    nc = bacc.Bacc(
        "TRN2",
        target_bir_lowering=False,
        debug=False,
        enable_asserts=False,
        num_devices=NCORES,
    )

    def din(name, shape, dt=F32):
        return nc.dram_tensor(name, list(shape), dt, kind="ExternalInput").ap()

    x2 = din("x2", (2, NLOC), F32R)            # rows: raw s, raw t (per-core slice)
    w0ts_d = din("w0ts", (2, H), F32R)         # (W0/std).T, std folded on host
    bt0 = din("bt0", (H,))                     # b0 - W0 @ (mean/std)
    wth = {l: din(f"w{l}th", (H, H), F16) for l in (1, 2, 3)}  # fp16 Wl.T
    w1wt = din("w1wt", (H, H), F16)     # (W1*diag(w0c1)).T fp16
    w1w2t = din("w1w2t", (H, H), F16)   # (W1*diag(-w0c1^2)).T fp16 (half conv)
    lt4h = din("lt4h", (6, 128, 3), F16)   # block-diag final lhsT, H2 col x2
    negid = din("negid", (128, 128), F16)  # -I for psum-accumulated subtraction
    bs = {l: din(f"b{l}", (H,)) for l in (1, 2, 3)}
    # host-folded PDE/tail scalars
    ct = din("ct", (1,))        # C
    nr_d = din("nr", (1,))      # -r
    c1_d = din("c1", (1,))      # -1/(K-C)
    mc3_d = din("mc3", (1,))    # 2r/(K-C)
    tmb_d = din("tmb", (1,))    # b4*tgt_std + tgt_mean
    sts_d = din("sts", (1,))    # tgt_std
    out = nc.dram_tensor("out", [3, NLOC], F32, kind="ExternalOutput").ap()

    def pool_mul(dst, a, b):
        nc.gpsimd.tensor_tensor(dst, a, b, OP.mult)

    def dve_mul(dst, a, b):
        nc.vector.tensor_tensor(dst, a, b, OP.mult)

    with tile.TileContext(nc) as tc:
        from contextlib import ExitStack

        with ExitStack() as ctx:
            const = ctx.enter_context(tc.tile_pool(name="const", bufs=1))
            sb = ctx.enter_context(tc.tile_pool(name="sb", bufs=1))
            ps = ctx.enter_context(tc.tile_pool(name="ps", bufs=1, space="PSUM"))

            # ---------- one-time prep ----------
            # DMA queue discipline: nc.sync is reserved for the x2/tp/out
            # streams so chunk-0's input issues immediately; const loads ride
            # the four engine queues in consumption order (L0-critical first,
            # tail-only scalars last).
            _qs = [nc.scalar]
            _qi = [0]

            def qdma(outt, inap):
                _qs[_qi[0] % len(_qs)].dma_start(out=outt, in_=inap)
                _qi[0] += 1

            def new1(name):
                return const.tile([128, 1], F32, name=name)

            def bc_tile2(src_ap, name):
                t = const.tile([128, 2], F32, name=name)
                qdma(t, bass.AP(src_ap.tensor, 0, [[0, 128], [1, 2]]))
                return t

            def bc_tile(src_ap, off, name):
                t = new1(name)
                qdma(t, bass.AP(src_ap.tensor, off, [[0, 128], [1, 1]]))
                return t

            # --- L0-critical loads (all folding done on host) ---
            w0ts = const.tile([2, H], F32R, name="w0ts")
            qdma(w0ts, w0ts_d)
            beta0 = []
            for m in range(2):
                t = new1(f"beta_{m}")
                qdma(t, bass.AP(bt0.tensor, 128 * m, [[1, 128], [1, 1]]))
                beta0.append(t)

            # --- prewarm the ACT function tables (Tanh/Copy/Square) on a
            # dummy tile so the 1.3us table loads overlap the input DMAs ---
            warm = const.tile([1, 1], F32, name="warm")
            nc.vector.memset(warm, 0.0)
            warm2 = const.tile([1, 1], F16, name="warm2")
            nc.scalar.activation(warm2, warm, AF.Tanh)

            # --- preload the first pair's inputs so they beat the bulk
            # const loads into the sync DMA queue ---
            x2_pre = {}
            for c in (0, 1):
                t = sb.tile([2, CH], F32R, tag="x2c", bufs=4)
                nc.sync.dma_start(out=t, in_=x2[:, c * CH : (c + 1) * CH])
                x2_pre[c] = t

            # --- hidden-layer weights: batched [128, 256] loads, layer order,
            # on the sync queue (ACT queue stays clear for tanh) ---
            def wload(name, srcd):
                halves = []
                for kk in range(2):
                    t = const.tile([128, H], F16, name=f"{name}_{kk}")
                    nc.sync.dma_start(
                        out=t,
                        in_=bass.AP(srcd.tensor, kk * 128 * H, [[H, 128], [1, H]]),
                    )
                    halves.append([t[:, 0:128], t[:, 128:256]])
                return halves

            bl = {}

            def bload(l):
                bl[l] = []
                for m in range(2):
                    t = new1(f"bl{l}_{m}")
                    nc.sync.dma_start(
                        out=t, in_=bass.AP(bs[l].tensor, 128 * m, [[1, 128], [1, 1]])
                    )
                    bl[l].append(t)

            wt16 = {}
            wtw = wload("wtw", w1wt)
            wtw2 = wload("wtw2", w1w2t)
            wt16[1] = wload("wth1", wth[1])
            bload(1)
            wt16[2] = wload("wth2", wth[2])
            bload(2)
            wt16[3] = wload("wth3", wth[3])
            bload(3)

            # final-projection block-diag lhsT: one [128, 18] load
            lt18 = const.tile([128, 18], F16, name="lt18")
            nc.sync.dma_start(
                out=lt18, in_=bass.AP(lt4h.tensor, 0, [[3, 128], [128 * 3, 6], [1, 3]])
            )
            lt4 = [
                [lt18[:, (2 * s + kk) * 3 : (2 * s + kk) * 3 + 3] for kk in range(2)]
                for s in range(3)
            ]
            nid = const.tile([128, 128], F16, name="nid")
            nc.sync.dma_start(out=nid, in_=negid)



            # ---------- main loop: chunks processed in interleaved pairs ----------
            # PE order alternates chunk A and B at each stage (L0..L3, FIN) so
            # every elementwise chain of one chunk is covered by the other
            # chunk's matmul stream; the PE never waits and stays at full
            # pstate. The -I/-h2t accumulations are deferred to the next
            # stage's flush point (always the other chunk's).
            y3f = sb.tile([3, NLOC], F32, name="y3f")
            tp = sb.tile([128, 3 * PPP], F32, name="tp")
            pend = []

            def flush_pend():
                while pend:
                    pend.pop(0)()

            def load_x2(c):
                x2c = sb.tile([2, CH], F32R, tag="x2c", bufs=4)
                nc.sync.dma_start(out=x2c, in_=x2[:, c * CH : (c + 1) * CH])
                return x2c

            def layer0(st_c, x2c):
                Hv, H1, H2 = st_c
                for m in range(2):
                    av = sb.tile([128, CH], F16, tag=f"hv{m}", bufs=4, name="av")
                    for g in range(NG):
                        sl = slice(g * PG, (g + 1) * PG)
                        pz0 = ps.tile([128, PG], F32, tag="pz0", bufs=2, name="pz0")
                        nc.tensor.matmul(
                            pz0,
                            w0ts[:, m * 128 : (m + 1) * 128],
                            x2c[:, sl],
                            start=True,
                            stop=True,
                        )
                        nc.scalar.activation(av[:, sl], pz0, AF.Tanh, beta0[m])
                    ee = sb.tile([128, CH], F16, tag=f"ee{m}", bufs=3, name="ee")
                    dve_mul(ee, av, av)
                    d0 = sb.tile([128, CH], F16, tag=f"h1{m}", bufs=4, name="d0")
                    nc.vector.tensor_scalar(d0, ee, -1.0, 1.0, OP.mult, OP.add)
                    ad = sb.tile([128, CH], F16, tag=f"h2{m}", bufs=4, name="ad")
                    dve_mul(ad, d0, av)
                    Hv[m], H1[m], H2[m] = av, d0, ad

            def layer(st_c, l):
                Hv, H1, H2 = st_c
                W1h = wtw if l == 1 else wt16[l]
                W2h = wtw2 if l == 1 else wt16[l]
                nHv = [None] * 2
                nH1 = [None] * 2
                nH2 = [None] * 2
                for m in range(2):
                    av = sb.tile([128, CH], F16, tag=f"hv{m}", bufs=4, name="av")
                    z1c = sb.tile([128, CH], F16, tag=f"z1c{m}", bufs=3, name="z1c")
                    for g in range(NG):
                        sl = slice(g * PG, (g + 1) * PG)
                        pz0 = ps.tile([128, PG], F32, tag="pz0", bufs=2, name="pz0")
                        for kk in range(2):
                            nc.tensor.matmul(
                                pz0,
                                wt16[l][kk][m],
                                Hv[kk][:, sl],
                                start=(kk == 0),
                                stop=(kk == 1),
                            )
                        nc.scalar.activation(av[:, sl], pz0, AF.Tanh, bl[l][m])
                    if m == 0:
                        flush_pend()
                    for g in range(NG):
                        sl = slice(g * PG, (g + 1) * PG)
                        pz1 = ps.tile([128, PG], F32, tag="pz1", bufs=2, name="pz1")
                        for kk in range(2):
                            nc.tensor.matmul(
                                pz1,
                                W1h[kk][m],
                                H1[kk][:, sl],
                                start=(kk == 0),
                                stop=(kk == 1),
                            )
                        nc.scalar.activation(z1c[:, sl], pz1, AF.Copy)
                    ee = sb.tile([128, CH], F16, tag=f"ee{m}", bufs=3, name="ee")
                    pool_mul(ee, av, av)
                    dm = sb.tile([128, CH], F16, tag=f"dm{m}", bufs=3, name="dm")
                    nc.vector.tensor_scalar(dm, ee, -1.0, 1.0, OP.mult, OP.add)
                    st = sb.tile([128, CH], F16, tag=f"st{m}", bufs=3, name="st")
                    if l == 1:
                        nc.scalar.activation(st, z1c, AF.Square)
                    else:
                        dve_mul(st, z1c, z1c)
                    tt = sb.tile([128, CH], F16, tag=f"tt{m}", bufs=3, name="tt")
                    dve_mul(tt, av, st)
                    h1t = sb.tile([128, CH], F16, tag=f"h1{m}", bufs=4, name="h1t")
                    dve_mul(h1t, dm, z1c)
                    # second-tangent partial sums; -tt matmul + evacuation
                    # deferred to the next stage's flush point. pz2 is a
                    # 2-bank tile so the evacuation runs 1024-wide.
                    h2t = sb.tile([128, CH], F16, tag=f"h2{m}", bufs=4, name="h2t")
                    pz2 = ps.tile([128, CH], F32, tag="pz2", bufs=2, name="pz2")
                    for g in range(NG):
                        sl = slice(g * PG, (g + 1) * PG)
                        for kk in range(2):
                            nc.tensor.matmul(
                                pz2[:, sl],
                                W2h[kk][m],
                                H2[kk][:, sl],
                                start=(kk == 0),
                                stop=False,
                            )

                    def fin(pz2=pz2, tt=tt, dm=dm, h2t=h2t):
                        for g in range(NG):
                            sl = slice(g * PG, (g + 1) * PG)
                            nc.tensor.matmul(
                                pz2[:, sl], nid, tt[:, sl], start=False, stop=True
                            )
                        nc.vector.tensor_tensor(h2t, pz2, dm, OP.mult)

                    pend.append(fin)
                    nHv[m], nH1[m], nH2[m] = av, h1t, h2t
                st_c[0], st_c[1], st_c[2] = nHv, nH1, nH2

            ypend = []

            def flush_ypend():
                while ypend:
                    ypend.pop(0)()

            def final_proj(st_c, c):
                Hv, H1, H2 = st_c
                for i in range(PT):
                    pyt = ps.tile([128, PG], F32, tag="pz1", bufs=2, name="pyt")
                    py = pyt[0:3, :]
                    first = True
                    for s_idx, stream in enumerate((Hv, H1, H2)):
                        for kk in range(2):
                            nc.tensor.matmul(
                                py,
                                lt4[s_idx][kk],
                                stream[kk][:, i * 512 : (i + 1) * 512],
                                start=first,
                                stop=(s_idx == 2 and kk == 1),
                            )
                            first = False
                    if i == 0:
                        flush_pend()

                    def ycopy(py=py, c=c, i=i):
                        nc.scalar.copy(
                            y3f[:, c * CH + i * 512 : c * CH + (i + 1) * 512], py
                        )
                        if i == PT - 1:
                            for s_idx in range(3):
                                nc.sync.dma_start(
                                    out=tp[c * 16 : (c + 1) * 16,
                                           s_idx * PPP : (s_idx + 1) * PPP],
                                    in_=y3f[s_idx : s_idx + 1,
                                            c * CH : (c + 1) * CH],
                                )

                    ypend.append(ycopy)

            # Software-pipelined pair loop. The next pair's L0 stages are
            # emitted between the FINs so the PE stream never drains at a
            # pair boundary.
            npairs = NCHUNK // 2
            nxt = {}
            for cp in range(npairs):
                A, Bc = 2 * cp, 2 * cp + 1
                for cn in (2 * cp + 2, 2 * cp + 3):
                    if cn < NCHUNK:
                        x2_pre[cn] = load_x2(cn)
                if cp == 0:
                    stA = [[None] * 2, [None] * 2, [None] * 2]
                    stB = [[None] * 2, [None] * 2, [None] * 2]
                    layer0(stA, x2_pre[A])
                    nc.scalar.activation(warm2, warm, AF.Copy)
                    nc.scalar.activation(warm2, warm, AF.Square)
                    layer0(stB, x2_pre[Bc])
                else:
                    stA, stB = nxt["A"], nxt["B"]
                for l in (1, 2, 3):
                    layer(stA, l)
                    layer(stB, l)
                final_proj(stA, A)
                if cp + 1 < npairs:
                    nA = [[None] * 2, [None] * 2, [None] * 2]
                    layer0(nA, x2_pre[2 * cp + 2])
                    nxt["A"] = nA
                    nB = [[None] * 2, [None] * 2, [None] * 2]
                    layer0(nB, x2_pre[2 * cp + 3])
                    nxt["B"] = nB
                final_proj(stB, Bc)
                flush_ypend()

            # --- tail scalars (host-folded) ---
            C_t = bc_tile(ct, 0, "bc_ct")
            nr = bc_tile(nr_d, 0, "bc_nr")
            c1 = bc_tile(c1_d, 0, "bc_c1")
            mc3 = bc_tile(mc3_d, 0, "bc_mc3")
            tmb = bc_tile(tmb_d, 0, "bc_tmb")
            sts = bc_tile(sts_d, 0, "bc_sts")
            bc_ts = sts

            # ----- tail (once): PDE algebra on the [128, PPP] layout -----
            yv = tp[:, 0:PPP]
            yt = tp[:, PPP : 2 * PPP]
            ytt = tp[:, 2 * PPP : 3 * PPP]
            oc = sb.tile([128, 3 * PPP], F32, name="oc")
            U = oc[:, 0:PPP]
            Fo = oc[:, PPP : 2 * PPP]
            Ft = oc[:, 2 * PPP : 3 * PPP]

            def tl(name):
                return sb.tile([128, PPP], F32, name=name)

            ut, utt, vv, v2, w1, q1, t1 = (
                tl("ut"), tl("utt"), tl("vv"), tl("v2"), tl("w1"), tl("q1"), tl("t1"),
            )
            nc.vector.tensor_scalar(U, yv, bc_ts, tmb, OP.mult, OP.add)
            nc.vector.tensor_scalar(ut, yt, sts, None, OP.mult)
            nc.vector.tensor_scalar(utt, ytt, sts, None, OP.mult)
            nc.vector.tensor_scalar(vv, U, C_t, None, OP.subtract)
            nc.vector.tensor_tensor(v2, vv, vv, OP.mult)
            nc.vector.scalar_tensor_tensor(w1, v2, c1, vv, OP.mult, OP.add)
            nc.vector.scalar_tensor_tensor(Fo, w1, nr, ut, OP.mult, OP.add)
            nc.vector.tensor_tensor(q1, vv, ut, OP.mult)
            nc.vector.scalar_tensor_tensor(t1, ut, nr, utt, OP.mult, OP.add)
            nc.vector.scalar_tensor_tensor(Ft, q1, mc3, t1, OP.mult, OP.add)
            for s_idx, srcap in enumerate((U, Fo, Ft)):
                (nc.sync if s_idx != 1 else nc.scalar).dma_start(
                    out=out[s_idx : s_idx + 1, :], in_=srcap
                )






    nc.compile()
    return nc


_STATE = {}


def _get_nc():
    if "nc" not in _STATE:
        _STATE["nc"] = _build()
    return _STATE["nc"]


def _make_lt4(w4):
    # final-projection block-diag lhsT; H2 stream carries h2/2, restore x2
    scales = (1.0, 1.0, 2.0)
    out = np.zeros((6, 128, 3), np.float32)
    for s_idx in range(3):
        for kk in range(2):
            out[2 * s_idx + kk, :, s_idx] = (
                scales[s_idx] * w4[0, kk * 128 : (kk + 1) * 128]
            )
    return out


def _prep_in_maps(inputs):
    f = np.float32

    def arr(k):
        return np.ascontiguousarray(np.asarray(inputs[k], f))

    x = np.asarray(inputs["inputs"], f).reshape(N, 2)
    istd = arr("in_std") + np.float32(1e-8)
    w0s = arr("W0") / istd[None, :]
    bt0 = arr("b0") - arr("W0") @ (arr("in_mean") / istd)
    r = np.exp(-arr("log_growth_rate"))
    K = np.float32(0.2) + np.float32(0.8) / (1 + np.exp(-arr("log_carrying_capacity")))
    C = np.float32(0.1) / (1 + np.exp(-arr("log_initial_loss")))
    ts1 = arr("tgt_std").reshape(1)
    shared = {
        "w0ts": np.ascontiguousarray(w0s.T),
        "bt0": np.ascontiguousarray(bt0),
        "lt4h": _make_lt4(arr("W4").reshape(1, H)).astype(np.float16),
        "w1th": np.ascontiguousarray(arr("W1").T).astype(np.float16),
        "w1wt": np.ascontiguousarray(
            (arr("W1") * arr("W0")[:, 1][None, :]).T
        ).astype(np.float16),
        "w1w2t": np.ascontiguousarray(
            (arr("W1") * (-(arr("W0")[:, 1] ** 2))[None, :]).T
        ).astype(np.float16),
        "negid": (-np.eye(128)).astype(np.float16),
        "w2th": np.ascontiguousarray(arr("W2").T).astype(np.float16),
        "w3th": np.ascontiguousarray(arr("W3").T).astype(np.float16),
        "b1": arr("b1"),
        "b2": arr("b2"),
        "b3": arr("b3"),
        "ct": np.asarray(C, f).reshape(1),
        "nr": np.asarray(-r, f).reshape(1),
        "c1": np.asarray(-1.0 / (K - C), f).reshape(1),
        "mc3": np.asarray(2.0 * r / (K - C), f).reshape(1),
        "tmb": np.asarray(
            arr("b4").reshape(1) * ts1 + arr("tgt_mean").reshape(1), f
        ).reshape(1),
        "sts": np.asarray(ts1, f).reshape(1),
    }
    in_maps = []
    for c in range(NCORES):
        m = dict(shared)
        m["x2"] = np.ascontiguousarray(x[c * NLOC : (c + 1) * NLOC].T)
        in_maps.append(m)
    return in_maps


def run(inputs, trace=False):
    nc = _get_nc()
    in_maps = _prep_in_maps(inputs)
    kw = {}
    if trace:
        kw["tmpdir"] = tempfile.mkdtemp(prefix="bassk_prof_")
    res = run_bass_kernel_spmd(
        nc, in_maps, core_ids=list(range(NCORES)), trace=trace, **kw
    )
    U = np.empty((N,), np.float32)
    F = np.empty((N,), np.float32)
    Ft = np.empty((N,), np.float32)
    for c in range(NCORES):
        o = res.results[c]["out"]
        U[c * NLOC : (c + 1) * NLOC] = o[0]
        F[c * NLOC : (c + 1) * NLOC] = o[1]
        Ft[c * NLOC : (c + 1) * NLOC] = o[2]
    shp = (B, S, 1)
    return (U.reshape(shp), F.reshape(shp), Ft.reshape(shp)), res


def kernel(**inputs):
    outs, _ = run(inputs, trace=False)
    return outs


# ---------------------------------------------------------------------------
# Dev-loop timing: persistent jitted executable (mirrors
# bass2jax.run_bass_via_pjrt's multi-core branch) so repeated executions
# reuse one compiled NEFF and can be timed back-to-back.
# ---------------------------------------------------------------------------
def _make_runner():
    if "runner" in _STATE:
        return _STATE["runner"]
    import jax
    from jax.experimental.shard_map import shard_map
    from jax.sharding import Mesh, PartitionSpec
    from concourse import bass2jax

    bass2jax.install_neuronx_cc_hook()
    nc = _get_nc()

    in_names, out_names, out_avals, zero_outs = [], [], [], []
    for alloc in nc.m.functions[0].allocations:
        if not isinstance(alloc, mybir.MemoryLocationSet):
            continue
        name = alloc.memorylocations[0].name
        if alloc.kind == "ExternalInput":
            if nc.partition_id_tensor is None or name != nc.partition_id_tensor.name:
                in_names.append(name)
        elif alloc.kind == "ExternalOutput":
            out_names.append(name)
            shape = tuple(alloc.tensor_shape)
            dtype = mybir.dt.np(alloc.dtype)
            out_avals.append(jax.core.ShapedArray(shape, dtype))
            zero_outs.append(np.zeros(shape, dtype))
    n_params = len(in_names)
    n_outs = len(out_avals)
    all_names = in_names + out_names
    if nc.partition_id_tensor is not None:
        all_names = all_names + [nc.partition_id_tensor.name]

    def _body(*args):
        operands = list(args)
        if nc.partition_id_tensor is not None:
            operands.append(bass2jax.partition_id_tensor())
        outs = bass2jax._bass_exec_p.bind(
            *operands,
            out_avals=tuple(out_avals),
            in_names=tuple(all_names),
            out_names=tuple(out_names),
            lowering_input_output_aliases=(),
            sim_require_finite=True,
            sim_require_nnan=True,
            nc=nc,
        )
        return tuple(outs)

    devices = jax.devices()[:NCORES]
    mesh = Mesh(np.asarray(devices), ("core",))
    donate = tuple(range(n_params, n_params + n_outs))
    sharded = jax.jit(
        shard_map(
            _body,
            mesh=mesh,
            in_specs=(PartitionSpec("core"),) * (n_params + n_outs),
            out_specs=(PartitionSpec("core"),) * n_outs,
            check_rep=False,
        ),
        donate_argnums=donate,
        keep_unused=True,
    )
    _STATE["runner"] = (sharded, in_names, out_names, out_avals, zero_outs)
    return _STATE["runner"]


def run_timed(inputs, iters=20):
    """Run via a persistent executable; return (outputs, per_iter_ns)."""
    import time as _time

    import jax

    sharded, in_names, out_names, out_avals, zero_outs = _make_runner()
    in_maps = _prep_in_maps(inputs)
    concat_in = [
        np.concatenate([np.asarray(in_maps[c][n]) for c in range(NCORES)], axis=0)
        for n in in_names
    ]
    dev_in = [jax.device_put(a) for a in concat_in]

    def zeros():
        return [
            np.zeros((NCORES * z.shape[0], *z.shape[1:]), z.dtype) for z in zero_outs
        ]

    # warmup (compiles on first call)
    outs = sharded(*dev_in, *zeros())
    jax.block_until_ready(outs)
    out_np = [np.asarray(o) for o in outs]

    zbufs = [zeros() for _ in range(iters)]
    t0 = _time.perf_counter()
    last = None
    for i in range(iters):
        last = sharded(*dev_in, *zbufs[i])
    jax.block_until_ready(last)
    t1 = _time.perf_counter()
    per_iter_ns = (t1 - t0) / iters * 1e9

    per_core = [
        {
            name: out_np[i].reshape(NCORES, *out_avals[i].shape)[c]
            for i, name in enumerate(out_names)
        }
        for c in range(NCORES)
    ]
    U = np.empty((N,), np.float32)
    F = np.empty((N,), np.float32)
    Ft = np.empty((N,), np.float32)
    for c in range(NCORES):
        o = per_core[c]["out"]
        U[c * NLOC : (c + 1) * NLOC] = o[0]
        F[c * NLOC : (c + 1) * NLOC] = o[1]
        Ft[c * NLOC : (c + 1) * NLOC] = o[2]
    shp = (B, S, 1)
    return (U.reshape(shp), F.reshape(shp), Ft.reshape(shp)), per_iter_ns
